# revision 5
# baseline (speedup 1.0000x reference)
"""HBV hydrological model (HBVMulTDET) Trainium2 Bass kernel.

Strategy:
  - Pure data parallelism: 4000 grid cells sharded as 500 cells/core x 8 cores.
  - Host precomputes all state-independent, forcing-dependent tensors in fp32
    with exact bit-equivalence to the reference ops:
      RAIN  = P * (T >= TT)
      SNOW  = P - RAIN
      PHI   = CFMAX*relu(dT) - CFR*CFMAX*relu(-dT)   (melt/refreeze are
              mutually exclusive so a single signed flux is exact)
      PETm  = PET broadcast over nmul
  - On-chip layout: [125 partitions = cell/4, free = (t, g=cell%4->4, nmul=8)]
    so every per-step elementwise op covers all 500*8 = 4000 local elements
    in a single instruction of free-size 32.
  - The only transcendental (soil wetness pow) runs on the Activation engine
    as exp(BETA*ln(SM) - BETA*ln(FC)); all other elementwise ops on DVE
    (Pool/GpSimd rejects TensorTensor opcodes on TRN2).
"""

import os
import sys

import numpy as np

for _p in ("/opt/trn_rl_repo",):
    if _p not in sys.path:
        sys.path.insert(0, _p)

T_FULL, G, NM = 730, 4000, 8
NCORES = 8
GL = G // NCORES          # 500 cells per core
P = 125                   # SBUF partitions used
GSUB = GL // P            # 4
FW = GSUB * NM            # 32 free elems per time step
NZ = 1e-5

BOUNDS = np.array([[1.0, 6.0], [50.0, 1000.0], [0.05, 0.9], [0.01, 0.5],
                   [0.001, 0.2], [0.2, 1.0], [0.0, 10.0], [0.0, 100.0],
                   [-2.5, 2.5], [0.5, 10.0], [0.0, 0.1], [0.0, 0.2]],
                  dtype=np.float32)

# const column order in the packed const tensor
_CONSTS = ["BETA", "LBF", "FC", "invLPFC", "PERCc", "UZL", "K0", "K1", "K2", "CWH"]
NCONST = len(_CONSTS)

_PROGRAM_CACHE = {}
LAST_RESULTS = None  # test.py reads exec_time_ns off this


def _build_program(t_steps, s_chunk):
    import concourse.bass as bass
    import concourse.bacc as bacc
    import concourse.mybir as mybir
    import concourse.tile as tile
    from contextlib import ExitStack

    f32 = mybir.dt.float32
    Alu = mybir.AluOpType
    Act = mybir.ActivationFunctionType

    nc = bacc.Bacc()

    d_snow = nc.dram_tensor("snow", [P, t_steps * FW], f32, kind="ExternalInput")
    d_rain = nc.dram_tensor("rain", [P, t_steps * FW], f32, kind="ExternalInput")
    d_phi = nc.dram_tensor("phi", [P, t_steps * FW], f32, kind="ExternalInput")
    d_pet = nc.dram_tensor("pet", [P, t_steps * FW], f32, kind="ExternalInput")
    d_const = nc.dram_tensor("consts", [P, NCONST * FW], f32, kind="ExternalInput")
    d_q = nc.dram_tensor("q", [P, t_steps * FW], f32, kind="ExternalOutput")

    chunks = []
    t0 = 0
    while t0 < t_steps:
        chunks.append((t0, min(s_chunk, t_steps - t0)))
        t0 += s_chunk

    VE, GE, AE = nc.vector, nc.gpsimd, nc.scalar

    with ExitStack() as ctx:
        tc = ctx.enter_context(tile.TileContext(nc))
        cpool = ctx.enter_context(tc.tile_pool(name="consts", bufs=1))
        spool = ctx.enter_context(tc.tile_pool(name="state", bufs=2))
        tpool = ctx.enter_context(tc.tile_pool(name="temps", bufs=2))
        ipool = ctx.enter_context(tc.tile_pool(name="inputs", bufs=2))
        opool = ctx.enter_context(tc.tile_pool(name="out", bufs=2))

        ct = cpool.tile([P, NCONST * FW], f32)
        nc.sync.dma_start(ct[:], d_const[:, :])
        C = {name: ct[:, i * FW:(i + 1) * FW] for i, name in enumerate(_CONSTS)}

        def st(tag):
            return tpool.tile([P, FW], f32, tag=tag, name=tag)

        # persistent states (tiles rotate; python vars track the live one)
        SP = spool.tile([P, FW], f32, tag="SP", name="SP")
        NMW = spool.tile([P, FW], f32, tag="NMW", name="NMW")   # negated meltwater
        SM = spool.tile([P, FW], f32, tag="SM", name="SM")
        SUZ = spool.tile([P, FW], f32, tag="SUZ", name="SUZ")
        SLZ = spool.tile([P, FW], f32, tag="SLZ", name="SLZ")
        VE.memset(SP[:], 0.001)
        VE.memset(NMW[:], -0.001)
        VE.memset(SM[:], 0.001)
        VE.memset(SUZ[:], 0.001)
        VE.memset(SLZ[:], 0.001)

        for (c0, clen) in chunks:
            cw_ = clen * FW
            snow_t = ipool.tile([P, cw_], f32, tag="snow", name="snow")
            rain_t = ipool.tile([P, cw_], f32, tag="rain", name="rain")
            phi_t = ipool.tile([P, cw_], f32, tag="phi", name="phi")
            pet_t = ipool.tile([P, cw_], f32, tag="pet", name="pet")
            cols = slice(c0 * FW, (c0 + clen) * FW)
            nc.sync.dma_start(snow_t[:], d_snow[:, cols])
            nc.sync.dma_start(rain_t[:], d_rain[:, cols])
            nc.sync.dma_start(phi_t[:], d_phi[:, cols])
            nc.sync.dma_start(pet_t[:], d_pet[:, cols])

            qout = opool.tile([P, cw_], f32, tag="qout", name="qout")

            for s in range(clen):
                sl = slice(s * FW, (s + 1) * FW)

                # ---- snow section (GpSimd) ----
                SP1 = st("SP1")
                VE.tensor_add(SP1[:], SP[:], snow_t[:, sl])
                mx = st("mx")
                VE.tensor_max(mx[:], phi_t[:, sl], NMW[:])
                net = st("net")
                VE.tensor_tensor(net[:], mx[:], SP1[:], Alu.min)
                SPn = spool.tile([P, FW], f32, tag="SP", name="SP")
                VE.tensor_sub(SPn[:], SP1[:], net[:])
                NMW2 = st("NMW2")
                VE.tensor_sub(NMW2[:], NMW[:], net[:])
                cw = st("cw")
                VE.tensor_mul(cw[:], C["CWH"], SPn[:])
                s6 = st("s6")
                VE.tensor_add(s6[:], NMW2[:], cw[:])
                q_ = st("q_")                       # q = -tosoil
                VE.tensor_scalar_min(q_[:], s6[:], 0.0)
                NMWn = spool.tile([P, FW], f32, tag="NMW", name="NMW")
                VE.tensor_sub(NMWn[:], NMW2[:], q_[:])
                SP, NMW = SPn, NMWn

                # ---- soil section (DVE + ACT) ----
                win = st("win")
                VE.tensor_sub(win[:], rain_t[:, sl], q_[:])
                lsm = st("lsm")
                AE.activation(lsm[:], SM[:], Act.Ln)
                e1 = st("e1")
                VE.tensor_mul(e1[:], C["BETA"], lsm[:])
                e2 = st("e2")
                VE.tensor_sub(e2[:], e1[:], C["LBF"])
                swe = st("swe")
                AE.activation(swe[:], e2[:], Act.Exp)
                sw = st("sw")
                VE.tensor_scalar_min(sw[:], swe[:], 1.0)
                rech = st("rech")
                VE.tensor_mul(rech[:], win[:], sw[:])
                SMa = st("SMa")
                VE.tensor_add(SMa[:], SM[:], win[:])
                SMb = st("SMb")
                VE.tensor_sub(SMb[:], SMa[:], rech[:])
                SMc = st("SMc")
                VE.tensor_tensor(SMc[:], SMb[:], C["FC"], Alu.min)
                exc = st("exc")
                VE.tensor_sub(exc[:], SMb[:], SMc[:])
                ef0 = st("ef0")
                VE.tensor_mul(ef0[:], SMc[:], C["invLPFC"])
                ef = st("ef")
                VE.tensor_scalar_min(ef[:], ef0[:], 1.0)
                etc = st("etc")
                VE.tensor_mul(etc[:], pet_t[:, sl], ef[:])
                eta = st("eta")
                VE.tensor_tensor(eta[:], SMc[:], etc[:], Alu.min)
                SMd = st("SMd")
                VE.tensor_sub(SMd[:], SMc[:], eta[:])
                SMn = spool.tile([P, FW], f32, tag="SM", name="SM")
                VE.tensor_scalar_max(SMn[:], SMd[:], NZ)
                SM = SMn

                # ---- response section (DVE) ----
                U1 = st("U1")
                VE.tensor_add(U1[:], SUZ[:], rech[:])
                U2 = st("U2")
                VE.tensor_add(U2[:], U1[:], exc[:])
                PERC = st("PERC")
                VE.tensor_tensor(PERC[:], U2[:], C["PERCc"], Alu.min)
                U3 = st("U3")
                VE.tensor_sub(U3[:], U2[:], PERC[:])
                u_ = st("u_")
                VE.tensor_sub(u_[:], U3[:], C["UZL"])
                ur = st("ur")
                VE.tensor_scalar_max(ur[:], u_[:], 0.0)
                Q0 = st("Q0")
                VE.tensor_mul(Q0[:], C["K0"], ur[:])
                U4 = spool.tile([P, FW], f32, tag="SUZ", name="SUZ")
                VE.tensor_sub(U4[:], U3[:], Q0[:])
                Q1 = st("Q1")
                VE.tensor_mul(Q1[:], C["K1"], U4[:])
                SUZn = spool.tile([P, FW], f32, tag="SUZ", name="SUZ")
                VE.tensor_sub(SUZn[:], U4[:], Q1[:])
                SUZ = SUZn
                SLZ1 = st("SLZ1")
                VE.tensor_add(SLZ1[:], SLZ[:], PERC[:])
                Q2 = st("Q2")
                VE.tensor_mul(Q2[:], C["K2"], SLZ1[:])
                SLZn = spool.tile([P, FW], f32, tag="SLZ", name="SLZ")
                VE.tensor_sub(SLZn[:], SLZ1[:], Q2[:])
                SLZ = SLZn
                qa = st("qa")
                VE.tensor_add(qa[:], Q0[:], Q1[:])
                VE.tensor_add(qout[:, sl], qa[:], Q2[:])

            nc.sync.dma_start(d_q[:, cols], qout[:])

    nc.finalize()
    return nc


def _to_kernel_layout(a, t_steps):
    # [T, GL, NM] -> [P, T*FW]  with cell_local = GSUB*p + g
    return np.ascontiguousarray(
        a.reshape(t_steps, P, GSUB, NM).transpose(1, 0, 2, 3).reshape(P, t_steps * FW)
    )


def _from_kernel_layout(a, t_steps):
    # [P, T*FW] -> [T, GL, NM]
    return a.reshape(P, t_steps, GSUB, NM).transpose(1, 0, 2, 3).reshape(t_steps, GL, NM)


def kernel(x_hydro_model, params_raw, t_steps=None):
    global LAST_RESULTS
    from concourse.bass_utils import run_bass_kernel_spmd

    if t_steps is None:
        t_steps = int(x_hydro_model.shape[0])
    s_chunk = int(os.environ.get("HBV_CHUNK", "73"))

    x = np.asarray(x_hydro_model, dtype=np.float32)
    pr = np.asarray(params_raw, dtype=np.float32)

    b = BOUNDS
    p = pr[-1] * (b[:, 1] - b[:, 0])[None, :, None] + b[:, 0][None, :, None]  # [G,12,NM]
    (BETA, FC, K0, K1, K2, LP, PERCc, UZL, TT, CFMAX, CFR, CWH) = (
        p[:, i, :] for i in range(12)
    )
    CFRX = CFR * CFMAX   # f32, matches (CFR*CFMAX) grouping in reference
    LBF = (BETA.astype(np.float64) * np.log(FC.astype(np.float64))).astype(np.float32)
    invLPFC = (1.0 / (LP.astype(np.float64) * FC.astype(np.float64))).astype(np.float32)

    in_maps = []
    for k in range(NCORES):
        cs = slice(k * GL, (k + 1) * GL)
        prcp = x[:t_steps, cs, 0]
        tmean = x[:t_steps, cs, 1]
        pet = x[:t_steps, cs, 2]
        dT = tmean[:, :, None] - TT[None, cs, :]            # [T, GL, NM]
        is_rain = (dT >= 0).astype(np.float32)
        RAIN = prcp[:, :, None] * is_rain
        SNOW = prcp[:, :, None] - RAIN
        PHI = CFMAX[None, cs, :] * np.maximum(dT, 0.0) - CFRX[None, cs, :] * np.maximum(-dT, 0.0)
        PETm = np.broadcast_to(pet[:, :, None], (t_steps, GL, NM)).astype(np.float32)

        consts = np.stack(
            [BETA[cs], LBF[cs], FC[cs], invLPFC[cs], PERCc[cs], UZL[cs], K0[cs],
             K1[cs], K2[cs], CWH[cs]], axis=0
        )  # [NCONST, GL, NM]
        consts_l = np.ascontiguousarray(
            consts.reshape(NCONST, P, GSUB, NM).transpose(1, 0, 2, 3).reshape(P, NCONST * FW)
        ).astype(np.float32)

        in_maps.append({
            "snow": _to_kernel_layout(SNOW.astype(np.float32), t_steps),
            "rain": _to_kernel_layout(RAIN.astype(np.float32), t_steps),
            "phi": _to_kernel_layout(PHI.astype(np.float32), t_steps),
            "pet": _to_kernel_layout(PETm, t_steps),
            "consts": consts_l,
        })

    key = (t_steps, s_chunk)
    if key not in _PROGRAM_CACHE:
        _PROGRAM_CACHE[key] = _build_program(t_steps, s_chunk)
    nc = _PROGRAM_CACHE[key]

    res = run_bass_kernel_spmd(nc, in_maps, core_ids=list(range(NCORES)))
    LAST_RESULTS = res

    out = np.concatenate(
        [_from_kernel_layout(res.results[k]["q"], t_steps) for k in range(NCORES)],
        axis=1,
    )
    return out.astype(np.float32)
